# revision 1
# baseline (speedup 1.0000x reference)
"""v3: grouped-level tables in bf16, one gather index per partition.

Host re-layout (free): three tables so each sample needs only 3 gathered
blocks instead of 22 rows:
  Lz  [2^20, 128]  bf16 : leaf rows of W (z vectors), 256B blocks
  G1  [2^8,  1024] bf16 : levels 1..8  packed per level-8 ancestor, 2KB blocks
  G2  [2^16, 1024] bf16 : levels 9..16 packed per level-16 ancestor, 2KB blocks
  GB  [2^20, 512]  bf16 : levels 17..20 packed per level-20 node, 1KB blocks
Root level 0 = W[0] for every sample -> broadcast once on chip.

Per 128-sample tile: 3 indirect DMAs (one idx per partition), then
products + tree-add reduction on DVE, sigmoid on ACT, product over levels.
"""

import sys

for _p in ("/opt/trn_rl_repo", "/root/.axon_site/_ro/trn_rl_repo"):
    if _p not in sys.path:
        sys.path.append(_p)

import ml_dtypes
import numpy as np

import concourse.bacc as bacc
import concourse.bass as bass
import concourse.mybir as mybir
import concourse.tile as tile
from concourse.bass_utils import run_bass_kernel_spmd

N_CORES = 8
BATCH = 65536
PER_CORE = BATCH // N_CORES        # 8192
DEPTH = 20
OFFSET = (1 << DEPTH) - 1
SIZE = (1 << (DEPTH + 1)) - 1
D = 128
P = 128
TILES = PER_CORE // P              # 64
NLEV = DEPTH + 1                   # 21
NA = 8                             # levels per G1/G2 block
NB = 4                             # levels 17..20 in GB
LROWS = 1 << DEPTH                 # 2^20
AROWS = 1 << 16
G1ROWS = 1 << 8

f32 = mybir.dt.float32
bf16 = mybir.dt.bfloat16
i32 = mybir.dt.int32
bfnp = ml_dtypes.bfloat16


def prepare_tables(W: np.ndarray):
    Wb = W.astype(bfnp)
    Lz = np.ascontiguousarray(Wb[OFFSET:OFFSET + LROWS])
    # G1: row r <-> level-8 node id c8 = r + 2^8 - 1; cols [l-1] = level l
    G1 = np.empty((G1ROWS, NA * D), dtype=bfnp)
    ids = np.arange(G1ROWS, dtype=np.int64) + (G1ROWS - 1)
    for lev in range(8, 0, -1):
        G1[:, (lev - 1) * D:lev * D] = Wb[ids]
        ids = (ids - 1) >> 1
    # G2: row r <-> level-16 node id c16 = r + 2^16 - 1; cols [l-9] = level l
    G2 = np.empty((AROWS, NA * D), dtype=bfnp)
    ids = np.arange(AROWS, dtype=np.int64) + (AROWS - 1)
    for lev in range(16, 8, -1):
        G2[:, (lev - 9) * D:(lev - 8) * D] = Wb[ids]
        ids = (ids - 1) >> 1
    # GB: row r <-> level-20 node id c20 = r + 2^20 - 1; cols [l-17] = level l
    GB = np.empty((LROWS, NB * D), dtype=bfnp)
    ids = np.arange(LROWS, dtype=np.int64) + (LROWS - 1)
    for lev in range(20, 16, -1):
        GB[:, (lev - 17) * D:(lev - 16) * D] = Wb[ids]
        ids = (ids - 1) >> 1
    w0 = np.broadcast_to(Wb[0:1], (P, D)).copy()
    return Lz, G1, G2, GB, w0


def build_kernel():
    nc = bacc.Bacc("TRN2", target_bir_lowering=False, debug=False,
                   num_devices=N_CORES)

    coll = nc.dram_tensor("coll", [PER_CORE, 2], i32, kind="ExternalInput")
    Lz = nc.dram_tensor("Lz", [LROWS, D], bf16, kind="ExternalInput")
    G1 = nc.dram_tensor("G1", [G1ROWS, NA * D], bf16, kind="ExternalInput")
    G2 = nc.dram_tensor("G2", [AROWS, NA * D], bf16, kind="ExternalInput")
    GB = nc.dram_tensor("GB", [LROWS, NB * D], bf16, kind="ExternalInput")
    w0 = nc.dram_tensor("w0", [P, D], bf16, kind="ExternalInput")
    out = nc.dram_tensor("out", [PER_CORE], f32, kind="ExternalOutput")

    with tile.TileContext(nc) as tc:
        with (
            tc.tile_pool(name="const", bufs=1) as cpool,
            tc.tile_pool(name="gz", bufs=4) as zpool,
            tc.tile_pool(name="ga", bufs=4) as apool,
            tc.tile_pool(name="gb", bufs=4) as bpool,
            tc.tile_pool(name="pr", bufs=2) as ppool,
            tc.tile_pool(name="ix", bufs=4) as ipool,
        ):
            coll_sb = cpool.tile([P, TILES, 2], i32)
            nc.sync.dma_start(
                out=coll_sb[:],
                in_=coll.ap().rearrange("(p n) c -> p n c", p=P),
            )
            w0_sb = cpool.tile([P, D], bf16)
            nc.sync.dma_start(out=w0_sb[:], in_=w0.ap())

            # idx_z = col0 ; idx_1 = (b>>12) - 2^8 ; idx_a = (b>>4) - 2^16 ;
            # idx_b = b - 2^20
            idx_z = cpool.tile([P, TILES], i32)
            idx_1 = cpool.tile([P, TILES], i32)
            idx_a = cpool.tile([P, TILES], i32)
            idx_b = cpool.tile([P, TILES], i32)
            b_sb = cpool.tile([P, TILES], i32)
            nc.vector.tensor_copy(out=idx_z[:], in_=coll_sb[:, :, 0])
            nc.vector.tensor_scalar(
                out=b_sb[:], in0=coll_sb[:, :, 1],
                scalar1=OFFSET + 1, scalar2=None, op0=mybir.AluOpType.add)
            nc.vector.tensor_scalar(
                out=idx_a[:], in0=b_sb[:], scalar1=4, scalar2=None,
                op0=mybir.AluOpType.logical_shift_right)
            nc.vector.tensor_scalar(
                out=idx_a[:], in0=idx_a[:], scalar1=AROWS, scalar2=None,
                op0=mybir.AluOpType.subtract)
            nc.vector.tensor_scalar(
                out=idx_1[:], in0=b_sb[:], scalar1=12, scalar2=None,
                op0=mybir.AluOpType.logical_shift_right)
            nc.vector.tensor_scalar(
                out=idx_1[:], in0=idx_1[:], scalar1=G1ROWS, scalar2=None,
                op0=mybir.AluOpType.subtract)
            nc.vector.tensor_scalar(
                out=idx_b[:], in0=b_sb[:], scalar1=LROWS, scalar2=None,
                op0=mybir.AluOpType.subtract)

            logits = cpool.tile([P, TILES, NLEV + 3], f32)

            for n in range(TILES):
                # stage this tile's indices into dedicated offset-0 [P,1]
                # tiles -- exactly the AP shape the production scatter_add
                # gather uses (nonzero-offset idx APs misbehave on HW)
                iz = ipool.tile([P, 1], i32, tag="iz", name="iz")
                i1 = ipool.tile([P, 1], i32, tag="i1", name="i1")
                ia = ipool.tile([P, 1], i32, tag="ia", name="ia")
                ib = ipool.tile([P, 1], i32, tag="ib", name="ib")
                nc.vector.tensor_copy(out=iz[:], in_=idx_z[:, n:n + 1])
                nc.vector.tensor_copy(out=i1[:], in_=idx_1[:, n:n + 1])
                nc.vector.tensor_copy(out=ia[:], in_=idx_a[:, n:n + 1])
                nc.vector.tensor_copy(out=ib[:], in_=idx_b[:, n:n + 1])
                gz = zpool.tile([P, D], bf16, tag="gz")
                g1 = apool.tile([P, NA * D], bf16, tag="g1", name="g1")
                ga = apool.tile([P, NA * D], bf16, tag="ga")
                gb = bpool.tile([P, NB * D], bf16, tag="gb")
                nc.gpsimd.indirect_dma_start(
                    out=gz[:], out_offset=None, in_=Lz.ap(),
                    in_offset=bass.IndirectOffsetOnAxis(ap=iz[:, :1], axis=0))
                nc.gpsimd.indirect_dma_start(
                    out=g1[:], out_offset=None, in_=G1.ap(),
                    in_offset=bass.IndirectOffsetOnAxis(ap=i1[:, :1], axis=0))
                nc.gpsimd.indirect_dma_start(
                    out=ga[:], out_offset=None, in_=G2.ap(),
                    in_offset=bass.IndirectOffsetOnAxis(ap=ia[:, :1], axis=0))
                nc.gpsimd.indirect_dma_start(
                    out=gb[:], out_offset=None, in_=GB.ap(),
                    in_offset=bass.IndirectOffsetOnAxis(ap=ib[:, :1], axis=0))

                z3 = gz[:].unsqueeze(1)  # [P,1,D]
                prod = ppool.tile([P, NLEV + 3, D], bf16, tag="prod")
                # levels 1..8
                nc.vector.tensor_tensor(
                    out=prod[:, 0:NA, :],
                    in0=g1[:].rearrange("p (l d) -> p l d", d=D),
                    in1=z3.to_broadcast([P, NA, D]),
                    op=mybir.AluOpType.mult)
                # levels 9..16
                nc.vector.tensor_tensor(
                    out=prod[:, NA:2 * NA, :],
                    in0=ga[:].rearrange("p (l d) -> p l d", d=D),
                    in1=z3.to_broadcast([P, NA, D]),
                    op=mybir.AluOpType.mult)
                # levels 17..20
                nc.vector.tensor_tensor(
                    out=prod[:, 2 * NA:2 * NA + NB, :],
                    in0=gb[:].rearrange("p (l d) -> p l d", d=D),
                    in1=z3.to_broadcast([P, NB, D]),
                    op=mybir.AluOpType.mult)
                # root (level 0)
                nc.vector.tensor_tensor(
                    out=prod[:, 2 * NA + NB, :],
                    in0=gz[:], in1=w0_sb[:],
                    op=mybir.AluOpType.mult)
                # pad rows so the tree-add works on 24 rows
                nc.vector.memset(prod[:, NLEV:, :], 0.0)
                # reduce over d: 3 halvings (128->16) then tensor_reduce
                h1 = ppool.tile([P, NLEV + 3, D // 2], bf16, tag="h1")
                nc.vector.tensor_tensor(
                    out=h1[:], in0=prod[:, :, 0:D // 2],
                    in1=prod[:, :, D // 2:D], op=mybir.AluOpType.add)
                h2 = ppool.tile([P, NLEV + 3, D // 4], bf16, tag="h2")
                nc.vector.tensor_tensor(
                    out=h2[:], in0=h1[:, :, 0:D // 4],
                    in1=h1[:, :, D // 4:D // 2], op=mybir.AluOpType.add)
                h3 = ppool.tile([P, NLEV + 3, D // 8], bf16, tag="h3")
                nc.vector.tensor_tensor(
                    out=h3[:], in0=h2[:, :, 0:D // 8],
                    in1=h2[:, :, D // 8:D // 4], op=mybir.AluOpType.add)
                nc.vector.tensor_reduce(
                    out=logits[:, n, :], in_=h3[:],
                    axis=mybir.AxisListType.X, op=mybir.AluOpType.add)

            # sigmoid + product over the 21 real levels
            sig = cpool.tile([P, TILES, 32], f32)
            nc.vector.memset(sig[:], 1.0)
            nc.scalar.activation(
                out=sig[:, :, 0:NLEV],
                in_=logits[:, :, 0:NLEV],
                func=mybir.ActivationFunctionType.Sigmoid)
            cur = sig
            width = 32
            while width > 2:
                width //= 2
                nxt = cpool.tile([P, TILES, width], f32, tag=f"tree{width}",
                                 name=f"tree{width}")
                nc.vector.tensor_tensor(
                    out=nxt[:], in0=cur[:, :, 0:width],
                    in1=cur[:, :, width:2 * width], op=mybir.AluOpType.mult)
                cur = nxt
            probs = cpool.tile([P, TILES], f32)
            nc.vector.tensor_tensor(
                out=probs[:], in0=cur[:, :, 0], in1=cur[:, :, 1],
                op=mybir.AluOpType.mult)
            nc.sync.dma_start(
                out=out.ap().rearrange("(p n) -> p n", p=P),
                in_=probs[:])

    nc.compile()
    return nc


_NC_CACHE = None


def _get_nc():
    global _NC_CACHE
    if _NC_CACHE is None:
        _NC_CACHE = build_kernel()
    return _NC_CACHE


def _run(collocation: np.ndarray, W: np.ndarray, trace: bool = False,
         **spmd_kwargs):
    collocation = np.ascontiguousarray(collocation, dtype=np.int32)
    W = np.ascontiguousarray(W, dtype=np.float32)
    assert collocation.shape == (BATCH, 2)
    assert W.shape == (SIZE, D)

    # Sort samples by context vertex so each gather instruction (one
    # 128-sample tile) touches 128 consecutive sorted samples -> ascending
    # DRAM addresses in the grouped tables. Device position q = p*64+j holds
    # sorted sample j*128+p (tile j = sorted samples [j*128,(j+1)*128)).
    order = np.argsort(collocation[:, 1], kind="stable").astype(np.int64)
    coll_sorted = collocation[order]
    arr = (np.arange(TILES)[None, :] * P
           + np.arange(P)[:, None]).reshape(-1)     # q -> local sorted idx

    Lz, G1, G2, GB, w0 = prepare_tables(W)
    nc = _get_nc()
    in_maps = []
    for c in range(N_CORES):
        core_sorted = coll_sorted[c * PER_CORE:(c + 1) * PER_CORE]
        in_maps.append(
            {"coll": np.ascontiguousarray(core_sorted[arr]),
             "Lz": Lz, "G1": G1, "G2": G2, "GB": GB, "w0": w0})
    res = run_bass_kernel_spmd(
        nc, in_maps, core_ids=list(range(N_CORES)), trace=trace,
        **spmd_kwargs)
    outs = []
    for c in range(N_CORES):
        od = res.results[c]["out"].reshape(P, TILES)
        outs.append(od.T.reshape(-1))               # back to sorted order
    out_sorted = np.concatenate(outs)
    out = np.empty_like(out_sorted)
    out[order] = out_sorted
    return out, res


def kernel(collocation: np.ndarray, W: np.ndarray) -> np.ndarray:
    out, _ = _run(collocation, W, trace=False)
    return out



# revision 3
# speedup vs baseline: 1.0143x; 1.0143x over previous
"""v10: static context-window sub-shards + batched dma_gather + TensorE top levels.

Layout (host, free):
  32 sub-shards by context window [s*2^15, (s+1)*2^15); core c runs shards
  4c..4c+3, each padded to C=2304 samples (18 tiles of 128).
  Tables (built once per W):
    G2 [2^16, 8*128]  bf16 : levels 9..16 keyed by level-16 ancestor (2KB rows)
    GB [2^20, 4*128]  bf16 : levels 17..20 keyed by leaf (1KB rows)
  Per-shard slices of G2 (2048 rows) / GB (32768 rows) are per-core inputs so
  gather indices are int16-relative.
  Levels 0..8: per-shard only 8 distinct level-8 ancestors -> 72-column dot
  table g1t [d=128, (9 levels)*(8 ancs)] on TensorE (zT stationary), selected
  with a host-built one-hot mask8.

Device per tile (128 samples):
  TensorE: psum[s,72] = zT^T @ g1t      (dots of z with all top-level rows)
  DVE: prod[s,12,128] = gathered rows * z ; reduce -> logits[s,9:21]
       topmul psum*mask8 ; reduce -> logits[s,0:9]
  ACT: sg[t] = sigmoid(logits) ; per shard: DVE product tree over 32-padded
  levels -> probs
"""

import sys

for _p in ("/opt/trn_rl_repo", "/root/.axon_site/_ro/trn_rl_repo"):
    if _p not in sys.path:
        sys.path.append(_p)

import ml_dtypes
import numpy as np

import concourse.bacc as bacc
import concourse.mybir as mybir
import concourse.tile as tile
from concourse.bass_utils import run_bass_kernel_spmd
from concourse.library_config import mlp

N_CORES = 8
BATCH = 65536
DEPTH = 20
OFFSET = (1 << DEPTH) - 1
SIZE = (1 << (DEPTH + 1)) - 1
D = 128
P = 128

NSHARD = 32
SPC = NSHARD // N_CORES          # shards per core = 4
SHW = 1 << 15                    # context window per shard
C = 2304                         # padded samples per shard
TPS = C // P                     # tiles per shard = 18
NCHUNK = 3                       # gather chunks per shard
TPCH = TPS // NCHUNK             # tiles per chunk = 6 (768-row gathers)
TPG = 6                          # tiles per top-level psum group
NGRP = TPS // TPG                # psum groups per shard = 3
ROWS_CH = TPCH * P               # rows per gather chunk = 768
G2ROWS = 1 << 11                 # G2 rows per shard
GBROWS = SHW                     # GB rows per shard
NL_G2 = 8                        # levels 9..16
NL_GB = 4                        # levels 17..20
NTOP = 9                         # levels 0..8
NANC8 = 8                        # level-8 ancestors per shard
TOPC = NTOP * NANC8              # 72 dot columns

f32 = mybir.dt.float32
bf16 = mybir.dt.bfloat16
i16 = mybir.dt.int16
bfnp = ml_dtypes.bfloat16


def build_tables(W: np.ndarray):
    Wb = W.astype(bfnp)
    G2 = np.empty((1 << 16, NL_G2 * D), dtype=bfnp)
    ids1 = np.arange(1 << 16, dtype=np.int64) + (1 << 16)   # 1-based level-16
    for lev in range(9, 17):
        G2[:, (lev - 9) * D:(lev - 8) * D] = Wb[(ids1 >> (16 - lev)) - 1]
    GB = np.empty((1 << 20, NL_GB * D), dtype=bfnp)
    idsB = np.arange(1 << 20, dtype=np.int64) + (1 << 20)   # 1-based leaf
    for lev in range(17, 21):
        GB[:, (lev - 17) * D:(lev - 16) * D] = Wb[(idsB >> (20 - lev)) - 1]
    return Wb, G2, GB


def wrap16(idx: np.ndarray) -> np.ndarray:
    """int16 row-gather index layout: idx i at [i%16, i//16], tiled to 128."""
    a = idx.astype(np.int16).reshape(-1, 16).T          # [16, n/16]
    return np.ascontiguousarray(np.tile(a, (8, 1)))     # [128, n/16]


def shard_inputs(Wb, G2, GB, ctx_sh, z0_sh, shard):
    """Build the 8 per-shard device inputs (ctx_sh/z0_sh already padded to C)."""
    b = ctx_sh.astype(np.int64) + (1 << 20)              # 1-based leaf ids
    relg2 = ((b >> 4) - (1 << 16) - shard * G2ROWS)
    relgb = (ctx_sh.astype(np.int64) - shard * SHW)
    assert relg2.min() >= 0 and relg2.max() < G2ROWS, "g2 idx oob"
    assert relgb.min() >= 0 and relgb.max() < GBROWS, "gb idx oob"
    r8 = ((b >> 12) - (1 << 8) - shard * NANC8)
    assert r8.min() >= 0 and r8.max() < NANC8
    m8 = np.zeros((C, NANC8), dtype=bfnp)
    m8[np.arange(C), r8] = 1
    z = Wb[z0_sh.astype(np.int64) + OFFSET]              # [C, 128]
    z3 = z.reshape(TPS, P, D)
    zp = np.ascontiguousarray(z3.transpose(1, 0, 2))     # [p, t, d]
    zt = np.ascontiguousarray(z3.transpose(2, 0, 1))     # [d, t, s]
    m8t = np.ascontiguousarray(m8.reshape(TPS, P, NANC8).transpose(1, 0, 2))
    # top-level dot table: col l*8+r = W[anc of (2^8 + 8*shard + r) at level l]
    nodes = np.empty(TOPC, dtype=np.int64)
    for lev in range(NTOP):
        for r in range(NANC8):
            if lev == 0:
                nodes[lev * NANC8 + r] = 0
            else:
                gid8 = (1 << 8) + NANC8 * shard + r
                nodes[lev * NANC8 + r] = (gid8 >> (8 - lev)) - 1
    g1t = np.ascontiguousarray(Wb[nodes].T)              # [d, 72]
    return {
        "g2": np.ascontiguousarray(G2[shard * G2ROWS:(shard + 1) * G2ROWS]),
        "gb": np.ascontiguousarray(GB[shard * GBROWS:(shard + 1) * GBROWS]),
        "zp": zp, "zt": zt, "m8": m8t, "g1t": g1t,
        "ig2": wrap16(relg2), "igb": wrap16(relgb),
    }


def build_kernel():
    nc = bacc.Bacc("TRN2", target_bir_lowering=False, debug=False,
                   num_devices=N_CORES, num_swdge_queues=4)

    ins = []
    for k in range(SPC):
        ins.append({
            "g2": nc.dram_tensor(f"g2_{k}", [G2ROWS, NL_G2 * D], bf16,
                                 kind="ExternalInput"),
            "gb": nc.dram_tensor(f"gb_{k}", [GBROWS, NL_GB * D], bf16,
                                 kind="ExternalInput"),
            "zp": nc.dram_tensor(f"zp_{k}", [P, TPS * D], bf16,
                                 kind="ExternalInput"),
            "zt": nc.dram_tensor(f"zt_{k}", [P, TPS * P], bf16,
                                 kind="ExternalInput"),
            "m8": nc.dram_tensor(f"m8_{k}", [P, TPS * NANC8], bf16,
                                 kind="ExternalInput"),
            "g1t": nc.dram_tensor(f"g1t_{k}", [P, TOPC], bf16,
                                  kind="ExternalInput"),
            "ig2": nc.dram_tensor(f"ig2_{k}", [P, C // 16], i16,
                                  kind="ExternalInput"),
            "igb": nc.dram_tensor(f"igb_{k}", [P, C // 16], i16,
                                  kind="ExternalInput"),
        })
    out = nc.dram_tensor("out", [P, SPC * TPS], f32, kind="ExternalOutput")

    with tile.TileContext(nc) as tc:
        with (
            tc.tile_pool(name="const", bufs=1) as cpool,
            tc.tile_pool(name="stream", bufs=2) as spool,
            tc.tile_pool(name="g2p", bufs=1) as gpool,
            tc.tile_pool(name="gbp", bufs=1) as bpool,
            tc.tile_pool(name="prodp", bufs=2) as ppool,
            tc.tile_pool(name="halfp", bufs=1) as hpool,
            tc.tile_pool(name="logp", bufs=2) as lpool,
            tc.tile_pool(name="psum", bufs=4, space="PSUM") as qpool,
        ):
            nc.gpsimd.load_library(mlp)
            probs = cpool.tile([P, SPC * TPS], f32)

            for k in range(SPC):
                t_in = ins[k]
                ig2 = spool.tile([P, C // 16], i16, tag="ig2")
                nc.scalar.dma_start(out=ig2[:], in_=t_in["ig2"].ap())
                igb = spool.tile([P, C // 16], i16, tag="igb")
                nc.scalar.dma_start(out=igb[:], in_=t_in["igb"].ap())
                zt = spool.tile([P, TPS, P], bf16, tag="zt")
                nc.sync.dma_start(out=zt[:], in_=t_in["zt"].ap().rearrange(
                    "p (t s) -> p t s", s=P))
                g1t = spool.tile([P, TOPC], bf16, tag="g1t")
                nc.sync.dma_start(out=g1t[:], in_=t_in["g1t"].ap())
                m8 = spool.tile([P, TPS, NANC8], bf16, tag="m8")
                nc.sync.dma_start(out=m8[:], in_=t_in["m8"].ap().rearrange(
                    "p (t r) -> p t r", r=NANC8))
                zp = spool.tile([P, TPS, D], bf16, tag="zp")
                nc.sync.dma_start(out=zp[:], in_=t_in["zp"].ap().rearrange(
                    "p (t d) -> p t d", d=D))

                lg = lpool.tile([P, TPS, 24], bf16, tag="lg")
                sg = lpool.tile([P, TPS, 32], f32, tag="sg")
                nc.vector.memset(sg[:, :, 21:32], 1.0)

                NLEV12 = NL_G2 + NL_GB
                for j in range(NCHUNK):
                    g2b = gpool.tile([P, TPCH, NL_G2 * D], bf16,
                                     tag=f"g2_{j}", name=f"g2b_{j}")
                    gbb = bpool.tile([P, TPCH, NL_GB * D], bf16,
                                     tag=f"gb_{j}", name=f"gbb_{j}")
                    cs = j * (ROWS_CH // 16)
                    gidx = (k * NCHUNK + j) * 2
                    nc.gpsimd.dma_gather(
                        g2b[:], t_in["g2"].ap(),
                        ig2[:, cs:cs + ROWS_CH // 16],
                        ROWS_CH, ROWS_CH, NL_G2 * D,
                        queue_num=gidx % 4)
                    nc.gpsimd.dma_gather(
                        gbb[:], t_in["gb"].ap(),
                        igb[:, cs:cs + ROWS_CH // 16],
                        ROWS_CH, ROWS_CH, NL_GB * D,
                        queue_num=(gidx + 1) % 4)

                    ts = j * TPCH
                    prod = ppool.tile([P, TPCH, NLEV12, D], bf16, tag="prod")
                    zc = zp[:, ts:ts + TPCH, :].unsqueeze(2)
                    nc.vector.tensor_tensor(
                        out=prod[:, :, 0:NL_G2, :],
                        in0=g2b[:].rearrange("p t (l d) -> p t l d", d=D),
                        in1=zc.to_broadcast([P, TPCH, NL_G2, D]),
                        op=mybir.AluOpType.mult)
                    nc.vector.tensor_tensor(
                        out=prod[:, :, NL_G2:, :],
                        in0=gbb[:].rearrange("p t (l d) -> p t l d", d=D),
                        in1=zc.to_broadcast([P, TPCH, NL_GB, D]),
                        op=mybir.AluOpType.mult)
                    cur, width = prod, D
                    while width > 8:
                        width //= 2
                        nxt = hpool.tile([P, TPCH, NLEV12, width], bf16,
                                         tag=f"ph{width}", name=f"ph{width}")
                        nc.vector.tensor_tensor(
                            out=nxt[:], in0=cur[:, :, :, 0:width],
                            in1=cur[:, :, :, width:2 * width],
                            op=mybir.AluOpType.add)
                        cur = nxt
                    with nc.allow_low_precision("bf16 logits are within tol"):
                        nc.vector.tensor_reduce(
                            out=lg[:, ts:ts + TPCH, NTOP:NTOP + NLEV12],
                            in_=cur[:], axis=mybir.AxisListType.X,
                            op=mybir.AluOpType.add)

                for g in range(NGRP):
                    pt = qpool.tile([P, TPG, TOPC], f32, tag="pt")
                    for i in range(TPG):
                        t = g * TPG + i
                        nc.tensor.matmul(pt[:, i, :], zt[:, t, :], g1t[:],
                                         start=True, stop=True)
                    ts = g * TPG
                    tm = hpool.tile([P, TPG, NTOP, NANC8], bf16, tag="tm")
                    nc.vector.tensor_tensor(
                        out=tm[:],
                        in0=pt[:].rearrange("p t (l r) -> p t l r", r=NANC8),
                        in1=m8[:, ts:ts + TPG, :].unsqueeze(2).to_broadcast(
                            [P, TPG, NTOP, NANC8]),
                        op=mybir.AluOpType.mult)
                    with nc.allow_low_precision("bf16 logits are within tol"):
                        nc.vector.tensor_reduce(
                            out=lg[:, ts:ts + TPG, 0:NTOP], in_=tm[:],
                            axis=mybir.AxisListType.X, op=mybir.AluOpType.add)

                nc.scalar.activation(
                    out=sg[:, :, 0:21], in_=lg[:, :, 0:21],
                    func=mybir.ActivationFunctionType.Sigmoid)

                cur, width = sg, 32
                while width > 2:
                    width //= 2
                    nxt = lpool.tile([P, TPS, width], f32, tag=f"h{width}",
                                     name=f"h{width}")
                    nc.vector.tensor_tensor(
                        out=nxt[:], in0=cur[:, :, 0:width],
                        in1=cur[:, :, width:2 * width], op=mybir.AluOpType.mult)
                    cur = nxt
                nc.vector.tensor_tensor(
                    out=probs[:, k * TPS:(k + 1) * TPS], in0=cur[:, :, 0],
                    in1=cur[:, :, 1], op=mybir.AluOpType.mult)

            nc.sync.dma_start(out=out.ap(), in_=probs[:])

    nc.compile()
    return nc


_NC_CACHE = None


def _get_nc():
    global _NC_CACHE
    if _NC_CACHE is None:
        _NC_CACHE = build_kernel()
    return _NC_CACHE


def _ref_probs(collocation, W, idx):
    """Exact numpy fallback for overflow samples (normally none)."""
    if len(idx) == 0:
        return np.zeros(0, dtype=np.float32)
    b = collocation[idx, 1].astype(np.int64) + OFFSET + 1
    z = W[collocation[idx, 0].astype(np.int64) + OFFSET]
    levels = np.arange(DEPTH + 1)
    path = (b[:, None] >> (DEPTH - levels)) - 1
    logits = np.einsum('bpd,bd->bp', W[path], z)
    return np.prod(1.0 / (1.0 + np.exp(-logits)), axis=-1).astype(np.float32)


def _run(collocation: np.ndarray, W: np.ndarray, trace: bool = False,
         **spmd_kwargs):
    collocation = np.ascontiguousarray(collocation, dtype=np.int32)
    W = np.ascontiguousarray(W, dtype=np.float32)
    assert collocation.shape == (BATCH, 2)
    assert W.shape == (SIZE, D)

    Wb, G2, GB = build_tables(W)
    ctx = collocation[:, 1].astype(np.int64)
    z0 = collocation[:, 0].astype(np.int64)
    order = np.argsort(ctx, kind="stable")
    ctx_s, z0_s = ctx[order], z0[order]
    bounds = np.searchsorted(ctx_s >> 15, np.arange(NSHARD + 1))

    nc = _get_nc()
    in_maps = []
    shard_n = []
    overflow_idx = []
    for c in range(N_CORES):
        m = {}
        for k in range(SPC):
            s = SPC * c + k
            st, en = bounds[s], bounds[s + 1]
            n = min(en - st, C)
            if en - st > C:
                overflow_idx.extend(order[st + C:en])
            shard_n.append(n)
            cpad = s * SHW + (np.arange(C, dtype=np.int64) * 2011) % SHW
            zpad = np.zeros(C, dtype=np.int64)
            cpad[:n] = ctx_s[st:st + n]
            zpad[:n] = z0_s[st:st + n]
            for name, arr in shard_inputs(Wb, G2, GB, cpad, zpad, s).items():
                m[f"{name}_{k}"] = arr
        in_maps.append(m)

    res = run_bass_kernel_spmd(
        nc, in_maps, core_ids=list(range(N_CORES)), trace=trace,
        **spmd_kwargs)

    out = np.empty(BATCH, dtype=np.float32)
    for c in range(N_CORES):
        oc = res.results[c]["out"]                       # [128, 72]
        for k in range(SPC):
            s = SPC * c + k
            n = shard_n[s]
            vals = oc[:, k * TPS:(k + 1) * TPS].T.reshape(C)   # sample t*128+p
            st = bounds[s]
            out[order[st:st + n]] = vals[:n]
    if overflow_idx:
        oi = np.asarray(overflow_idx, dtype=np.int64)
        out[oi] = _ref_probs(collocation, W, oi)
    return out, res


def kernel(collocation: np.ndarray, W: np.ndarray) -> np.ndarray:
    out, _ = _run(collocation, W, trace=False)
    return out
